# revision 24
# baseline (speedup 1.0000x reference)
"""Trainium2 Bass kernel: multi-scale masked average-pool descriptors.

Computes, per batch element b and scribble i:
    d_l[b,i,c] = mean over {pixels where resize(scribble)[b,i,y,x] > 0.5} of feat_l[b,c,y,x]
    out[b,i,c] = (d_0 + d_1 + d_2) / 3

Strategy (v6 -- trace-driven rework of the measured v4 design):
  * jax.image.resize(bilinear, antialias=False) at scales 4/8/16 reduces to an
    exact 2x2 average at stride k with offset o (k,o) = (4,1)/(8,3)/(16,7):
    mask == ((a+c)+(b+d)) > 2.0 bit-exactly in fp32 (computed on DVE).
  * ALL loads (features AND scribbles) ride the two HWDGE rings, strictly
    alternating.  The dma-issuing engines (sync/scalar-ACT) carry as little
    compute as possible so the rings never starve; GpSimd carries NO dma
    issuance at all (the v4 SWDGE scribble path serialized Q7 descriptor
    generation with everything else gpsimd did).
  * Feature maps use FULL-ROW descriptors ([y, c-group, x] tiles) -- the
    descriptor walk does the [c,y,x] -> [y,...] partition transpose for free.
    L1/L2 loads carry 2 c-groups per instruction (fewer, larger instructions).
  * The [y,c,x] -> [y,x,256c] bf16 repack copies are split over THREE engines
    weighted by their measured strided-copy rates: DVE 1.31 / ACT 2.35 /
    GpSimd 3.5 ns per free-elem  ->  9 / 4 / 3 of each level's 16 groups.
  * L0 scribbles are front-loaded so msk0 completes early, and the L0 matmul
    is split into two c-half PSUM banks: the lo bank only needs c-groups 0-7,
    so the PE starts at ~half L0 assembly instead of waiting for all of it.
  * cnt[i] folds into the mask compare via accum_out (per-row counts land in
    r[li] for free); a ones-matmul then reduces over rows.  bf16 masks are
    exact 0/1 and PSUM accumulates fp32, so cnt is exact and masks match the
    reference bit-exactly.  bf16 features give rel err ~2e-3 (gate: 2e-2).
  * Each level's staged [ssum | cnt] block is DMA'd out as soon as that
    level's matmuls finish (3 small stores instead of one trailing one).
  * The empty-mask fallback is handled on the host (P(empty) ~ 2^-1024).

Sharding: pure data-parallel over batch B=8 across the 8 NeuronCores.
"""

import numpy as np

_B = 8
_I = 16
_C = 256

# level: (h, k, off)
_LEVELS = {0: (128, 4, 1), 1: (64, 8, 3), 2: (32, 16, 7)}

# repack-copy engine per 16-channel group, weighted by measured strided-copy
# rates (DVE 1.8 / ACT 2.4 / GpSimd 3.8 ns per free-elem under concurrency).
# L0 loads LAST: its final groups sit on DVE (fastest) so the hi-bank matmuls
# start as soon as possible after the last load.
_ACT_GROUPS_L0 = {0, 2, 8, 10}
_GP_GROUPS_L0 = {1, 3, 12}
_ACT_GROUPS_L12 = {0, 3, 6, 9, 12, 15}
_GP_GROUPS_L12 = {2, 5, 8, 11}


def _build_nc():
    import concourse.bacc as bacc
    import concourse.tile as tile
    from concourse import mybir

    f32 = mybir.dt.float32
    bf16 = mybir.dt.bfloat16
    gt = mybir.AluOpType.is_gt
    X = mybir.AxisListType.X

    nc = bacc.Bacc("TRN2", target_bir_lowering=False, debug=False)

    feats = {
        0: nc.dram_tensor("feat0", [_C, 128, 128], f32, kind="ExternalInput"),
        1: nc.dram_tensor("feat1", [_C, 64, 64], f32, kind="ExternalInput"),
        2: nc.dram_tensor("feat2", [_C, 32, 32], f32, kind="ExternalInput"),
    }
    scr = nc.dram_tensor("scribbles", [_I, 512, 512], f32, kind="ExternalInput")
    out_d = nc.dram_tensor("out", [_I, 3 * (_C + 1)], f32, kind="ExternalOutput")

    with tile.TileContext(nc) as tc:
        with (
            tc.tile_pool(name="singles", bufs=1) as singles,
            tc.tile_pool(name="scrib", bufs=2) as scrib,
            tc.tile_pool(name="scrib2", bufs=3) as scrib2,
            tc.tile_pool(name="tmp", bufs=2) as tmp,
            tc.tile_pool(name="fR", bufs=5) as fR,
            tc.tile_pool(name="psumA", bufs=1, space="PSUM") as psumA,
        ):
            stag = singles.tile([_I, 3 * (_C + 1)], f32, tag="stag")

            # masks, y-on-partitions (natural resize layout): msk_l[y, i, x]
            msk0 = singles.tile([128, _I, 128], bf16, tag="msk0")
            msk1 = singles.tile([64, _I, 64], bf16, tag="msk1")
            msk2 = singles.tile([32, _I, 32], bf16, tag="msk2")
            msk = {0: msk0, 1: msk1, 2: msk2}
            # assembled feature tiles [y, x, c | ones] bf16: column 256 is
            # constant 1.0 so the accumulating matmuls produce cnt for free
            sgT0 = singles.tile([128, 128, _C + 1], bf16, tag="sgT0")
            sgT1 = singles.tile([64, 64, _C + 1], bf16, tag="sgT1")
            sgT2 = singles.tile([32, 32, _C + 1], bf16, tag="sgT2")
            sgT = {0: sgT0, 1: sgT1, 2: sgT2}
            for _sg in (sgT0, sgT1, sgT2):
                nc.vector.memset(_sg[:, :, _C : _C + 1], 1.0)

            # strict ring alternation for every dma_start
            rc = [0]

            def ring():
                eng = nc.sync if rc[0] % 2 == 0 else nc.scalar
                rc[0] += 1
                return eng

            def copy_to(li, g, srcv):
                dst = sgT[li][:, :, 16 * g : 16 * (g + 1)]
                act_g = _ACT_GROUPS_L0 if li == 0 else _ACT_GROUPS_L12
                gp_g = _GP_GROUPS_L0 if li == 0 else _GP_GROUPS_L12
                if g in act_g:
                    nc.scalar.copy(dst, srcv)
                elif g in gp_g:
                    nc.gpsimd.tensor_copy(dst, srcv)
                else:
                    nc.vector.tensor_copy(dst, srcv)

            def mask_ops(li, i, st, il=None):
                # only the 2-of-k needed resize columns are added (strided);
                # the compare's accum_out drops the per-row count into rs[li]
                h, k, off = _LEVELS[li]
                src_lo = st[:, il, 0:512] if il is not None else st[:, 0, :]
                src_hi = st[:, il, 512:1024] if il is not None else st[:, 1, :]
                a = src_lo.rearrange("p (x k) -> p x k", k=k)[:, :, off : off + 2]
                b = src_hi.rearrange("p (x k) -> p x k", k=k)[:, :, off : off + 2]
                v = tmp.tile([h, h, 2], f32, tag="v")
                nc.vector.tensor_add(v[:], a, b)
                sr = tmp.tile([h, h], f32, tag="sr")
                nc.vector.tensor_add(sr[:], v[:, :, 0], v[:, :, 1])
                nc.vector.tensor_scalar(
                    out=msk[li][:, i, :], in0=sr[:], scalar1=2.0,
                    scalar2=None, op0=gt,
                )

            def feat_load_l0(g):
                sg = fR.tile([128, 16, 128], f32, tag="sgR")
                ring().dma_start(
                    out=sg[:],
                    in_=feats[0][16 * g : 16 * (g + 1)].rearrange(
                        "c y x -> y c x"
                    ),
                )
                copy_to(0, g, sg[:].rearrange("p c x -> p x c"))

            def feat_load2(li, gg):
                # 2 c-groups per dma instruction on levels 1/2
                h = _LEVELS[li][0]
                sg = fR.tile([h, 32, h], f32, tag="sgR")
                ring().dma_start(
                    out=sg[:],
                    in_=feats[li][32 * gg : 32 * (gg + 1)].rearrange(
                        "c y x -> y c x"
                    ),
                )
                for sub in range(2):
                    g = 2 * gg + sub
                    srcv = sg[:, 16 * sub : 16 * (sub + 1), :].rearrange(
                        "p c x -> p x c"
                    )
                    copy_to(li, g, srcv)

            def scrib_load(li, i):
                h, k, off = _LEVELS[li]
                rrk = 512 // h
                st = scrib2.tile([h, 2, 512], f32, tag=f"st{li}")
                ring().dma_start(
                    out=st[:],
                    in_=scr[i].rearrange("(y r) x -> y r x", r=rrk)[
                        :, off : off + 2, :
                    ],
                )
                mask_ops(li, i, st)

            # ---- phase A: ALL scribbles + L1/L2 features interleaved ------
            # L1/L2 scribbles all land in the first 4 iterations so rr1/rr2
            # can be computed mid-phase without ever head-of-line-blocking
            # later DVE work; L0's 16 MB streams afterwards while the PE
            # chews on L1/L2.
            for t in range(8):
                i0 = t * 2
                st = scrib.tile([128, 2, 1024], f32, tag="st0")
                ring().dma_start(
                    out=st[:],
                    in_=scr[i0 : i0 + 2]
                    .rearrange("i (y k) x -> y i k x", k=4)[:, :, 1:3, :]
                    .rearrange("y i k x -> y i (k x)"),
                )
                feat_load2(1, t)
                mask_ops(0, i0, st, il=0)
                mask_ops(0, i0 + 1, st, il=1)
                scrib_load(1, i0)
                scrib_load(2, i0)
                feat_load2(2, t)
                scrib_load(1, i0 + 1)
                scrib_load(2, i0 + 1)

            # ---- phase B: L0 features (the bulk) --------------------------
            for g in range(16):
                feat_load_l0(g)

            # ---- matmuls: L0 in two c-half banks (lo needs groups 0-7 only),
            # then L1, L2.  Each level's staging block stores out immediately.
            def level_out(li, acc_aps):
                base = li * (_C + 1)
                off = 0
                for ap in acc_aps:
                    w = ap.shape[-1]
                    nc.vector.tensor_copy(stag[:, base + off : base + off + w], ap)
                    off += w
                ring().dma_start(
                    out=out_d[:, base : base + _C + 1],
                    in_=stag[:, base : base + _C + 1],
                )

            # PE emission order matches data readiness: L1, L2 (mid-kernel),
            # then L0 as two c-half banks (lo needs only c-groups 0-7)
            for li in (1, 2):
                h = _LEVELS[li][0]
                acc = psumA.tile([_I, _C + 1], f32, tag="acc")
                for x in range(h):
                    nc.tensor.matmul(
                        acc[:], msk[li][:, :, x], sgT[li][:, x, :],
                        start=(x == 0), stop=(x == h - 1),
                    )
                level_out(li, [acc[:]])

            acc_lo = psumA.tile([_I, 128], f32, tag="acc_lo")
            acc_hi = psumA.tile([_I, 129], f32, tag="acc_hi")
            for x in range(128):
                nc.tensor.matmul(
                    acc_lo[:], msk0[:, :, x], sgT0[:, x, 0:128],
                    start=(x == 0), stop=(x == 127),
                )
            for x in range(128):
                nc.tensor.matmul(
                    acc_hi[:], msk0[:, :, x], sgT0[:, x, 128:257],
                    start=(x == 0), stop=(x == 127),
                )
            level_out(0, [acc_lo[:], acc_hi[:]])

    nc.compile()
    return nc


def _host_fallback(scr_bi, fmap_b, h, k, off):
    """Feature at argmax of the soft mask; only used when a mask is empty."""
    V = scr_bi[off::k, :][:h].astype(np.float32) + scr_bi[off + 1 :: k, :][:h]
    sr4 = V[:, off::k][:, :h] + V[:, off + 1 :: k][:, :h]
    idx = int(np.argmax(np.float32(0.25) * sr4))
    y, x = divmod(idx, h)
    return fmap_b[:, y, x]


def _host_cnt(scribbles):
    """Exact per-level foreground-pixel counts, [B, I, 3].  The kernel's mask
    math is bit-identical, so any mismatch means the device returned garbage
    (wedged NeuronCore) and the run must be retried."""
    out = np.empty((scribbles.shape[0], scribbles.shape[1], 3), np.float32)
    for li, (h, k, off) in _LEVELS.items():
        V = (
            scribbles[:, :, off::k, :][:, :, :h].astype(np.float32)
            + scribbles[:, :, off + 1 :: k, :][:, :, :h]
        )
        S = V[:, :, :, off::k][:, :, :, :h] + V[:, :, :, off + 1 :: k][:, :, :, :h]
        out[:, :, li] = (S > np.float32(2.0)).sum(axis=(-1, -2))
    return out


def kernel(feat0, feat1, feat2, scribbles):
    import sys

    for p in ("/opt/trn_rl_repo", "/opt/pypackages"):
        if p not in sys.path:
            sys.path.append(p)
    from concourse.bass_utils import run_bass_kernel_spmd

    feat0 = np.asarray(feat0, dtype=np.float32)
    feat1 = np.asarray(feat1, dtype=np.float32)
    feat2 = np.asarray(feat2, dtype=np.float32)
    scribbles = np.asarray(scribbles, dtype=np.float32)

    nc = _build_nc()
    in_maps = [
        {
            "feat0": np.ascontiguousarray(feat0[b]),
            "feat1": np.ascontiguousarray(feat1[b]),
            "feat2": np.ascontiguousarray(feat2[b]),
            "scribbles": np.ascontiguousarray(scribbles[b]),
        }
        for b in range(_B)
    ]
    want_cnt = _host_cnt(scribbles)
    for attempt in range(3):
        res = run_bass_kernel_spmd(nc, in_maps, core_ids=list(range(_B)))
        raw = np.stack([res.results[b]["out"] for b in range(_B)])  # [B,I,3*257]
        raw = raw.reshape(_B, _I, 3, _C + 1)
        ssum = raw[..., :_C].astype(np.float32)  # [B, I, 3, C]
        cnt = raw[..., _C].astype(np.float32)  # [B, I, 3]
        if np.array_equal(cnt, want_cnt) and np.isfinite(ssum).all():
            break
        # device returned garbage (wedged core) -- rebuild and retry
        nc = _build_nc()

    mean = ssum / np.maximum(cnt, np.float32(1.0))[..., None]

    if (cnt == 0).any():  # never for non-degenerate inputs
        fm = [feat0, feat1, feat2]
        for b, i, li in zip(*np.nonzero(cnt == 0)):
            h, k, off = _LEVELS[li]
            mean[b, i, li] = _host_fallback(scribbles[b, i], fm[li][b], h, k, off)

    out = (mean[:, :, 0] + mean[:, :, 1] + mean[:, :, 2]) / np.float32(3.0)
    return out.astype(np.float32)


# revision 25
# speedup vs baseline: 1.0368x; 1.0368x over previous
"""Trainium2 Bass kernel: multi-scale masked average-pool descriptors.

Computes, per batch element b and scribble i:
    d_l[b,i,c] = mean over {pixels where resize(scribble)[b,i,y,x] > 0.5} of feat_l[b,c,y,x]
    out[b,i,c] = (d_0 + d_1 + d_2) / 3

Strategy (v6 -- trace-driven rework of the measured v4 design):
  * jax.image.resize(bilinear, antialias=False) at scales 4/8/16 reduces to an
    exact 2x2 average at stride k with offset o (k,o) = (4,1)/(8,3)/(16,7):
    mask == ((a+c)+(b+d)) > 2.0 bit-exactly in fp32 (computed on DVE).
  * ALL loads (features AND scribbles) ride the two HWDGE rings, strictly
    alternating.  The dma-issuing engines (sync/scalar-ACT) carry as little
    compute as possible so the rings never starve; GpSimd carries NO dma
    issuance at all (the v4 SWDGE scribble path serialized Q7 descriptor
    generation with everything else gpsimd did).
  * Feature maps use FULL-ROW descriptors ([y, c-group, x] tiles) -- the
    descriptor walk does the [c,y,x] -> [y,...] partition transpose for free.
    L1/L2 loads carry 2 c-groups per instruction (fewer, larger instructions).
  * The [y,c,x] -> [y,x,256c] bf16 repack copies are split over THREE engines
    weighted by their measured strided-copy rates: DVE 1.31 / ACT 2.35 /
    GpSimd 3.5 ns per free-elem  ->  9 / 4 / 3 of each level's 16 groups.
  * L0 scribbles are front-loaded so msk0 completes early, and the L0 matmul
    is split into two c-half PSUM banks: the lo bank only needs c-groups 0-7,
    so the PE starts at ~half L0 assembly instead of waiting for all of it.
  * cnt[i] folds into the mask compare via accum_out (per-row counts land in
    r[li] for free); a ones-matmul then reduces over rows.  bf16 masks are
    exact 0/1 and PSUM accumulates fp32, so cnt is exact and masks match the
    reference bit-exactly.  bf16 features give rel err ~2e-3 (gate: 2e-2).
  * Each level's staged [ssum | cnt] block is DMA'd out as soon as that
    level's matmuls finish (3 small stores instead of one trailing one).
  * The empty-mask fallback is handled on the host (P(empty) ~ 2^-1024).

Sharding: pure data-parallel over batch B=8 across the 8 NeuronCores.
"""

import numpy as np

_B = 8
_I = 16
_C = 256

# level: (h, k, off)
_LEVELS = {0: (128, 4, 1), 1: (64, 8, 3), 2: (32, 16, 7)}

# repack-copy engine per 16-channel group, weighted by measured strided-copy
# rates (DVE 1.8 / ACT 2.4 / GpSimd 3.8 ns per free-elem under concurrency).
# L0 loads LAST: its final groups sit on DVE (fastest) so the hi-bank matmuls
# start as soon as possible after the last load.
_ACT_GROUPS_L0 = {0, 2, 4, 6}
_GP_GROUPS_L0 = {1, 3, 5}
_ACT_GROUPS_L12 = {0, 3, 6, 9, 12, 15}
_GP_GROUPS_L12 = {2, 5, 8, 11}


def _build_nc():
    import concourse.bacc as bacc
    import concourse.tile as tile
    from concourse import mybir

    f32 = mybir.dt.float32
    bf16 = mybir.dt.bfloat16
    gt = mybir.AluOpType.is_gt
    X = mybir.AxisListType.X

    nc = bacc.Bacc("TRN2", target_bir_lowering=False, debug=False)

    feats = {
        0: nc.dram_tensor("feat0", [_C, 128, 128], f32, kind="ExternalInput"),
        1: nc.dram_tensor("feat1", [_C, 64, 64], f32, kind="ExternalInput"),
        2: nc.dram_tensor("feat2", [_C, 32, 32], f32, kind="ExternalInput"),
    }
    scr = nc.dram_tensor("scribbles", [_I, 512, 512], f32, kind="ExternalInput")
    out_d = nc.dram_tensor("out", [_I, 3 * (_C + 1)], f32, kind="ExternalOutput")

    with tile.TileContext(nc) as tc:
        with (
            tc.tile_pool(name="singles", bufs=1) as singles,
            tc.tile_pool(name="scrib", bufs=2) as scrib,
            tc.tile_pool(name="scrib2", bufs=3) as scrib2,
            tc.tile_pool(name="tmp", bufs=2) as tmp,
            tc.tile_pool(name="fR", bufs=5) as fR,
            tc.tile_pool(name="psumA", bufs=1, space="PSUM") as psumA,
            tc.tile_pool(name="psumB", bufs=2, space="PSUM") as psumB,
        ):
            stag = singles.tile([_I, 3 * (_C + 1)], f32, tag="stag")

            # masks, y-on-partitions (natural resize layout): msk_l[y, i, x]
            msk0 = singles.tile([128, _I, 128], bf16, tag="msk0")
            msk1 = singles.tile([64, _I, 64], bf16, tag="msk1")
            msk2 = singles.tile([32, _I, 32], bf16, tag="msk2")
            msk = {0: msk0, 1: msk1, 2: msk2}
            # assembled feature tiles [y, x, c] bf16 (256-wide: the row
            # stride must stay 32B-aligned -- a 257-wide tile measurably
            # slows every repack write and rhs read)
            sgT0 = singles.tile([128, 128, _C], bf16, tag="sgT0")
            sgT1 = singles.tile([64, 64, _C], bf16, tag="sgT1")
            sgT2 = singles.tile([32, 32, _C], bf16, tag="sgT2")
            sgT = {0: sgT0, 1: sgT1, 2: sgT2}
            # per-row mask counts, written by reduce_sum on DVE
            rr0 = singles.tile([128, _I], f32, tag="r0")
            rr1 = singles.tile([64, _I], f32, tag="r1")
            rr2 = singles.tile([32, _I], f32, tag="r2")
            rs = {0: rr0, 1: rr1, 2: rr2}
            ones = singles.tile([128, 1], f32, tag="ones")
            nc.vector.memset(ones[:], 1.0)

            # strict ring alternation for every dma_start
            rc = [0]

            def ring():
                eng = nc.sync if rc[0] % 2 == 0 else nc.scalar
                rc[0] += 1
                return eng

            def copy_to(li, g, srcv):
                dst = sgT[li][:, :, 16 * g : 16 * (g + 1)]
                act_g = _ACT_GROUPS_L0 if li == 0 else _ACT_GROUPS_L12
                gp_g = _GP_GROUPS_L0 if li == 0 else _GP_GROUPS_L12
                if g in act_g:
                    nc.scalar.copy(dst, srcv)
                elif g in gp_g:
                    nc.gpsimd.tensor_copy(dst, srcv)
                else:
                    nc.vector.tensor_copy(dst, srcv)

            def mask_ops(li, i, st, il=None):
                # only the 2-of-k needed resize columns are added (strided);
                # the compare's accum_out drops the per-row count into rs[li]
                h, k, off = _LEVELS[li]
                src_lo = st[:, il, 0:512] if il is not None else st[:, 0, :]
                src_hi = st[:, il, 512:1024] if il is not None else st[:, 1, :]
                a = src_lo.rearrange("p (x k) -> p x k", k=k)[:, :, off : off + 2]
                b = src_hi.rearrange("p (x k) -> p x k", k=k)[:, :, off : off + 2]
                v = tmp.tile([h, h, 2], f32, tag="v")
                nc.vector.tensor_add(v[:], a, b)
                sr = tmp.tile([h, h], f32, tag="sr")
                nc.vector.tensor_add(sr[:], v[:, :, 0], v[:, :, 1])
                nc.vector.tensor_scalar(
                    out=msk[li][:, i, :], in0=sr[:], scalar1=2.0,
                    scalar2=None, op0=gt,
                )

            def feat_load_l0(g):
                sg = fR.tile([128, 16, 128], f32, tag="sgR")
                ring().dma_start(
                    out=sg[:],
                    in_=feats[0][16 * g : 16 * (g + 1)].rearrange(
                        "c y x -> y c x"
                    ),
                )
                copy_to(0, g, sg[:].rearrange("p c x -> p x c"))

            def feat_load2(li, gg):
                # 2 c-groups per dma instruction on levels 1/2
                h = _LEVELS[li][0]
                sg = fR.tile([h, 32, h], f32, tag="sgR")
                ring().dma_start(
                    out=sg[:],
                    in_=feats[li][32 * gg : 32 * (gg + 1)].rearrange(
                        "c y x -> y c x"
                    ),
                )
                for sub in range(2):
                    g = 2 * gg + sub
                    srcv = sg[:, 16 * sub : 16 * (sub + 1), :].rearrange(
                        "p c x -> p x c"
                    )
                    copy_to(li, g, srcv)

            def scrib_load(li, i):
                h, k, off = _LEVELS[li]
                rrk = 512 // h
                st = scrib2.tile([h, 2, 512], f32, tag=f"st{li}")
                ring().dma_start(
                    out=st[:],
                    in_=scr[i].rearrange("(y r) x -> y r x", r=rrk)[
                        :, off : off + 2, :
                    ],
                )
                mask_ops(li, i, st)

            # ---- phase A: ALL scribbles + L1/L2 features interleaved ------
            # L1/L2 scribbles all land in the first 4 iterations so rr1/rr2
            # can be computed mid-phase without ever head-of-line-blocking
            # later DVE work; L0's 16 MB streams afterwards while the PE
            # chews on L1/L2.
            for t in range(8):
                i0 = t * 2
                st = scrib.tile([128, 2, 1024], f32, tag="st0")
                ring().dma_start(
                    out=st[:],
                    in_=scr[i0 : i0 + 2]
                    .rearrange("i (y k) x -> y i k x", k=4)[:, :, 1:3, :]
                    .rearrange("y i k x -> y i (k x)"),
                )
                feat_load2(1, t)
                mask_ops(0, i0, st, il=0)
                mask_ops(0, i0 + 1, st, il=1)
                scrib_load(1, i0)
                scrib_load(2, i0)
                feat_load2(2, t)
                scrib_load(1, i0 + 1)
                scrib_load(2, i0 + 1)

            nc.vector.reduce_sum(out=rr1[:], in_=msk1[:], axis=X)
            nc.vector.reduce_sum(out=rr2[:], in_=msk2[:], axis=X)

            # ---- phase B: L0 features (the bulk) --------------------------
            for g in range(16):
                feat_load_l0(g)

            # rr0 feeds only the very last cnt matmul; emitting it after the
            # L0 copies keeps the DVE FIFO clear for them
            nc.vector.reduce_sum(out=rr0[:], in_=msk0[:], axis=X)

            # ---- matmuls: L0 in two c-half banks (lo needs groups 0-7 only),
            # then L1, L2.  Each level's staging block stores out immediately.
            def level_out(li, acc_aps, cnt_src):
                base = li * (_C + 1)
                off = 0
                for ap in acc_aps:
                    w = ap.shape[-1]
                    nc.vector.tensor_copy(stag[:, base + off : base + off + w], ap)
                    off += w
                nc.vector.tensor_copy(
                    stag[:, base + _C : base + _C + 1], cnt_src
                )
                ring().dma_start(
                    out=out_d[:, base : base + _C + 1],
                    in_=stag[:, base : base + _C + 1],
                )

            # PE emission order matches data readiness: L1, L2 (mid-kernel),
            # then L0 as two c-half banks (lo needs only c-groups 0-7)
            for li in (1, 2):
                h = _LEVELS[li][0]
                acc = psumA.tile([_I, _C], f32, tag="acc")
                for x in range(h):
                    nc.tensor.matmul(
                        acc[:], msk[li][:, :, x], sgT[li][:, x, :],
                        start=(x == 0), stop=(x == h - 1),
                    )
                cnt = psumB.tile([_I, 1], f32, tag="cnt")
                nc.tensor.matmul(
                    cnt[:], rs[li][:], ones[: _LEVELS[li][0], :],
                    start=True, stop=True,
                )
                level_out(li, [acc[:]], cnt[:])

            acc_lo = psumA.tile([_I, 128], f32, tag="acc_lo")
            acc_hi = psumA.tile([_I, 128], f32, tag="acc_hi")
            for x in range(128):
                nc.tensor.matmul(
                    acc_lo[:], msk0[:, :, x], sgT0[:, x, 0:128],
                    start=(x == 0), stop=(x == 127),
                )
            for x in range(128):
                nc.tensor.matmul(
                    acc_hi[:], msk0[:, :, x], sgT0[:, x, 128:256],
                    start=(x == 0), stop=(x == 127),
                )
            cnt0 = psumB.tile([_I, 1], f32, tag="cnt")
            nc.tensor.matmul(cnt0[:], rr0[:], ones[:128, :], start=True, stop=True)
            level_out(0, [acc_lo[:], acc_hi[:]], cnt0[:])

    nc.compile()
    return nc


def _host_fallback(scr_bi, fmap_b, h, k, off):
    """Feature at argmax of the soft mask; only used when a mask is empty."""
    V = scr_bi[off::k, :][:h].astype(np.float32) + scr_bi[off + 1 :: k, :][:h]
    sr4 = V[:, off::k][:, :h] + V[:, off + 1 :: k][:, :h]
    idx = int(np.argmax(np.float32(0.25) * sr4))
    y, x = divmod(idx, h)
    return fmap_b[:, y, x]


def _host_cnt(scribbles):
    """Exact per-level foreground-pixel counts, [B, I, 3].  The kernel's mask
    math is bit-identical, so any mismatch means the device returned garbage
    (wedged NeuronCore) and the run must be retried."""
    out = np.empty((scribbles.shape[0], scribbles.shape[1], 3), np.float32)
    for li, (h, k, off) in _LEVELS.items():
        V = (
            scribbles[:, :, off::k, :][:, :, :h].astype(np.float32)
            + scribbles[:, :, off + 1 :: k, :][:, :, :h]
        )
        S = V[:, :, :, off::k][:, :, :, :h] + V[:, :, :, off + 1 :: k][:, :, :, :h]
        out[:, :, li] = (S > np.float32(2.0)).sum(axis=(-1, -2))
    return out


def kernel(feat0, feat1, feat2, scribbles):
    import sys

    for p in ("/opt/trn_rl_repo", "/opt/pypackages"):
        if p not in sys.path:
            sys.path.append(p)
    from concourse.bass_utils import run_bass_kernel_spmd

    feat0 = np.asarray(feat0, dtype=np.float32)
    feat1 = np.asarray(feat1, dtype=np.float32)
    feat2 = np.asarray(feat2, dtype=np.float32)
    scribbles = np.asarray(scribbles, dtype=np.float32)

    nc = _build_nc()
    in_maps = [
        {
            "feat0": np.ascontiguousarray(feat0[b]),
            "feat1": np.ascontiguousarray(feat1[b]),
            "feat2": np.ascontiguousarray(feat2[b]),
            "scribbles": np.ascontiguousarray(scribbles[b]),
        }
        for b in range(_B)
    ]
    want_cnt = _host_cnt(scribbles)
    for attempt in range(3):
        res = run_bass_kernel_spmd(nc, in_maps, core_ids=list(range(_B)))
        raw = np.stack([res.results[b]["out"] for b in range(_B)])  # [B,I,3*257]
        raw = raw.reshape(_B, _I, 3, _C + 1)
        ssum = raw[..., :_C].astype(np.float32)  # [B, I, 3, C]
        cnt = raw[..., _C].astype(np.float32)  # [B, I, 3]
        if np.array_equal(cnt, want_cnt) and np.isfinite(ssum).all():
            break
        # device returned garbage (wedged core) -- rebuild and retry
        nc = _build_nc()

    mean = ssum / np.maximum(cnt, np.float32(1.0))[..., None]

    if (cnt == 0).any():  # never for non-degenerate inputs
        fm = [feat0, feat1, feat2]
        for b, i, li in zip(*np.nonzero(cnt == 0)):
            h, k, off = _LEVELS[li]
            mean[b, i, li] = _host_fallback(scribbles[b, i], fm[li][b], h, k, off)

    out = (mean[:, :, 0] + mean[:, :, 1] + mean[:, :, 2]) / np.float32(3.0)
    return out.astype(np.float32)


# revision 26
# speedup vs baseline: 1.0498x; 1.0126x over previous
"""Trainium2 Bass kernel: multi-scale masked average-pool descriptors.

Computes, per batch element b and scribble i:
    d_l[b,i,c] = mean over {pixels where resize(scribble)[b,i,y,x] > 0.5} of feat_l[b,c,y,x]
    out[b,i,c] = (d_0 + d_1 + d_2) / 3

Strategy (v12 -- trace-driven rework of the measured v4 design):
  * jax.image.resize(bilinear, antialias=False) at scales 4/8/16 reduces to an
    exact 2x2 average at stride k with offset o (k,o) = (4,1)/(8,3)/(16,7):
    mask == ((a+c)+(b+d)) > 2.0 bit-exactly in fp32 (computed on DVE).
  * ALL loads (features AND scribbles) ride the two HWDGE rings, strictly
    alternating.  The dma-issuing engines (sync/scalar-ACT) carry as little
    compute as possible so the rings never starve; GpSimd carries NO dma
    issuance at all (the v4 SWDGE scribble path serialized Q7 descriptor
    generation with everything else gpsimd did).
  * Feature maps use FULL-ROW descriptors ([y, c-group, x] tiles) -- the
    descriptor walk does the [c,y,x] -> [y,...] partition transpose for free.
    L1/L2 loads carry 2 c-groups per instruction (fewer, larger instructions).
  * The [y,c,x] -> [y,x,256c] bf16 repack copies are split over THREE engines
    weighted by their measured strided-copy rates: DVE 1.31 / ACT 2.35 /
    GpSimd 3.5 ns per free-elem  ->  9 / 4 / 3 of each level's 16 groups.
  * L0 scribbles are front-loaded so msk0 completes early, and the L0 matmul
    is split into two c-half PSUM banks: the lo bank only needs c-groups 0-7,
    so the PE starts at ~half L0 assembly instead of waiting for all of it.
  * cnt[i] = reduce_sum over the mask rows + a ones-matmul; bf16 masks are
    exact 0/1 and PSUM accumulates fp32, so cnt is exact and masks match the
    reference bit-exactly.  bf16 features give rel err ~2e-3 (gate: 2e-2).
    The reduces are placed so they never head-of-line-block the DVE FIFO
    ahead of repack copies (rr0 only feeds the final cnt matmul).
  * Each level's staged [ssum | cnt] block is DMA'd out as soon as that
    level's matmuls finish (3 small stores instead of one trailing one).
  * The empty-mask fallback is handled on the host (P(empty) ~ 2^-1024).

Sharding: pure data-parallel over batch B=8 across the 8 NeuronCores.
"""

import numpy as np

_B = 8
_I = 16
_C = 256

# level: (h, k, off)
_LEVELS = {0: (128, 4, 1), 1: (64, 8, 3), 2: (32, 16, 7)}

# repack-copy engine per 16-channel group, weighted by measured strided-copy
# rates (DVE 1.8 / ACT 2.4 / GpSimd 3.8 ns per free-elem under concurrency).
# L0 loads LAST: its final groups sit on DVE (fastest) so the hi-bank matmuls
# start as soon as possible after the last load.
_ACT_GROUPS_L0 = {0, 2, 4, 6}
_GP_GROUPS_L0 = {1, 3, 5}
_ACT_GROUPS_L12 = {0, 3, 6, 9, 12, 15}
_GP_GROUPS_L12 = {2, 5, 8, 11}


def _build_nc():
    import concourse.bacc as bacc
    import concourse.tile as tile
    from concourse import mybir

    f32 = mybir.dt.float32
    bf16 = mybir.dt.bfloat16
    gt = mybir.AluOpType.is_gt
    X = mybir.AxisListType.X

    nc = bacc.Bacc("TRN2", target_bir_lowering=False, debug=False)

    feats = {
        0: nc.dram_tensor("feat0", [_C, 128, 128], f32, kind="ExternalInput"),
        1: nc.dram_tensor("feat1", [_C, 64, 64], f32, kind="ExternalInput"),
        2: nc.dram_tensor("feat2", [_C, 32, 32], f32, kind="ExternalInput"),
    }
    scr = nc.dram_tensor("scribbles", [_I, 512, 512], f32, kind="ExternalInput")
    out_d = nc.dram_tensor("out", [_I, 3 * (_C + 1)], f32, kind="ExternalOutput")

    with tile.TileContext(nc) as tc:
        with (
            tc.tile_pool(name="singles", bufs=1) as singles,
            tc.tile_pool(name="scrib", bufs=2) as scrib,
            tc.tile_pool(name="scrib2", bufs=3) as scrib2,
            tc.tile_pool(name="tmp", bufs=2) as tmp,
            tc.tile_pool(name="fR", bufs=5) as fR,
            tc.tile_pool(name="psumA", bufs=1, space="PSUM") as psumA,
            tc.tile_pool(name="psumB", bufs=2, space="PSUM") as psumB,
        ):
            stag = singles.tile([_I, 3 * (_C + 1)], f32, tag="stag")

            # masks, y-on-partitions (natural resize layout): msk_l[y, i, x]
            msk0 = singles.tile([128, _I, 128], bf16, tag="msk0")
            msk1 = singles.tile([64, _I, 64], bf16, tag="msk1")
            msk2 = singles.tile([32, _I, 32], bf16, tag="msk2")
            msk = {0: msk0, 1: msk1, 2: msk2}
            # assembled feature tiles [y, x, c] bf16 (256-wide: the row
            # stride must stay 32B-aligned -- a 257-wide tile measurably
            # slows every repack write and rhs read)
            sgT0 = singles.tile([128, 128, _C], bf16, tag="sgT0")
            sgT1 = singles.tile([64, 64, _C], bf16, tag="sgT1")
            sgT2 = singles.tile([32, 32, _C], bf16, tag="sgT2")
            sgT = {0: sgT0, 1: sgT1, 2: sgT2}
            # per-row mask counts, written by reduce_sum on DVE
            rr0 = singles.tile([128, _I], f32, tag="r0")
            rr1 = singles.tile([64, _I], f32, tag="r1")
            rr2 = singles.tile([32, _I], f32, tag="r2")
            rs = {0: rr0, 1: rr1, 2: rr2}
            ones = singles.tile([128, 1], f32, tag="ones")
            nc.vector.memset(ones[:], 1.0)

            # strict ring alternation for every dma_start
            rc = [0]

            def ring():
                eng = nc.sync if rc[0] % 2 == 0 else nc.scalar
                rc[0] += 1
                return eng

            def copy_to(li, g, srcv):
                dst = sgT[li][:, :, 16 * g : 16 * (g + 1)]
                act_g = _ACT_GROUPS_L0 if li == 0 else _ACT_GROUPS_L12
                gp_g = _GP_GROUPS_L0 if li == 0 else _GP_GROUPS_L12
                if g in act_g:
                    nc.scalar.copy(dst, srcv)
                elif g in gp_g:
                    nc.gpsimd.tensor_copy(dst, srcv)
                else:
                    nc.vector.tensor_copy(dst, srcv)

            def mask_ops(li, i, st, il=None):
                # only the 2-of-k needed resize columns are added (strided)
                h, k, off = _LEVELS[li]
                src_lo = st[:, il, 0:512] if il is not None else st[:, 0, :]
                src_hi = st[:, il, 512:1024] if il is not None else st[:, 1, :]
                a = src_lo.rearrange("p (x k) -> p x k", k=k)[:, :, off : off + 2]
                b = src_hi.rearrange("p (x k) -> p x k", k=k)[:, :, off : off + 2]
                v = tmp.tile([h, h, 2], f32, tag="v")
                nc.vector.tensor_add(v[:], a, b)
                sr = tmp.tile([h, h], f32, tag="sr")
                nc.vector.tensor_add(sr[:], v[:, :, 0], v[:, :, 1])
                nc.vector.tensor_scalar(
                    out=msk[li][:, i, :], in0=sr[:], scalar1=2.0,
                    scalar2=None, op0=gt,
                )

            def feat_load_l0(g):
                sg = fR.tile([128, 16, 128], f32, tag="sgR")
                ring().dma_start(
                    out=sg[:],
                    in_=feats[0][16 * g : 16 * (g + 1)].rearrange(
                        "c y x -> y c x"
                    ),
                )
                copy_to(0, g, sg[:].rearrange("p c x -> p x c"))

            def feat_load2(li, gg):
                # 2 c-groups per dma instruction on levels 1/2
                h = _LEVELS[li][0]
                sg = fR.tile([h, 32, h], f32, tag="sgR")
                ring().dma_start(
                    out=sg[:],
                    in_=feats[li][32 * gg : 32 * (gg + 1)].rearrange(
                        "c y x -> y c x"
                    ),
                )
                for sub in range(2):
                    g = 2 * gg + sub
                    srcv = sg[:, 16 * sub : 16 * (sub + 1), :].rearrange(
                        "p c x -> p x c"
                    )
                    copy_to(li, g, srcv)

            def scrib_load(li, i):
                h, k, off = _LEVELS[li]
                rrk = 512 // h
                st = scrib2.tile([h, 2, 512], f32, tag=f"st{li}")
                ring().dma_start(
                    out=st[:],
                    in_=scr[i].rearrange("(y r) x -> y r x", r=rrk)[
                        :, off : off + 2, :
                    ],
                )
                mask_ops(li, i, st)

            # ---- phase A: ALL scribbles + L1/L2 features interleaved ------
            # L1/L2 scribbles all land in the first 4 iterations so rr1/rr2
            # can be computed mid-phase without ever head-of-line-blocking
            # later DVE work; L0's 16 MB streams afterwards while the PE
            # chews on L1/L2.
            for t in range(8):
                i0 = t * 2
                st = scrib.tile([128, 2, 1024], f32, tag="st0")
                ring().dma_start(
                    out=st[:],
                    in_=scr[i0 : i0 + 2]
                    .rearrange("i (y k) x -> y i k x", k=4)[:, :, 1:3, :]
                    .rearrange("y i k x -> y i (k x)"),
                )
                feat_load2(1, t)
                mask_ops(0, i0, st, il=0)
                mask_ops(0, i0 + 1, st, il=1)
                scrib_load(1, i0)
                scrib_load(2, i0)
                feat_load2(2, t)
                scrib_load(1, i0 + 1)
                scrib_load(2, i0 + 1)

            nc.vector.reduce_sum(out=rr1[:], in_=msk1[:], axis=X)
            nc.vector.reduce_sum(out=rr2[:], in_=msk2[:], axis=X)

            # ---- phase B: L0 features (the bulk) --------------------------
            for g in range(16):
                feat_load_l0(g)

            # rr0 feeds only the very last cnt matmul; emitting it after the
            # L0 copies keeps the DVE FIFO clear for them
            nc.vector.reduce_sum(out=rr0[:], in_=msk0[:], axis=X)

            # ---- matmuls: L0 in two c-half banks (lo needs groups 0-7 only),
            # then L1, L2.  Each level's staging block stores out immediately.
            def level_out(li, acc_aps, cnt_src):
                base = li * (_C + 1)
                off = 0
                for ap in acc_aps:
                    w = ap.shape[-1]
                    nc.vector.tensor_copy(stag[:, base + off : base + off + w], ap)
                    off += w
                nc.vector.tensor_copy(
                    stag[:, base + _C : base + _C + 1], cnt_src
                )
                ring().dma_start(
                    out=out_d[:, base : base + _C + 1],
                    in_=stag[:, base : base + _C + 1],
                )

            # PE emission order matches data readiness: L1, L2 (mid-kernel),
            # then L0 as two c-half banks (lo needs only c-groups 0-7)
            for li in (1, 2):
                h = _LEVELS[li][0]
                acc = psumA.tile([_I, _C], f32, tag="acc")
                for x in range(h):
                    nc.tensor.matmul(
                        acc[:], msk[li][:, :, x], sgT[li][:, x, :],
                        start=(x == 0), stop=(x == h - 1),
                    )
                cnt = psumB.tile([_I, 1], f32, tag="cnt")
                nc.tensor.matmul(
                    cnt[:], rs[li][:], ones[: _LEVELS[li][0], :],
                    start=True, stop=True,
                )
                level_out(li, [acc[:]], cnt[:])

            acc_lo = psumA.tile([_I, 128], f32, tag="acc_lo")
            acc_hi = psumA.tile([_I, 128], f32, tag="acc_hi")
            for x in range(128):
                nc.tensor.matmul(
                    acc_lo[:], msk0[:, :, x], sgT0[:, x, 0:128],
                    start=(x == 0), stop=(x == 127),
                )
            for x in range(128):
                nc.tensor.matmul(
                    acc_hi[:], msk0[:, :, x], sgT0[:, x, 128:256],
                    start=(x == 0), stop=(x == 127),
                )
            cnt0 = psumB.tile([_I, 1], f32, tag="cnt")
            nc.tensor.matmul(cnt0[:], rr0[:], ones[:128, :], start=True, stop=True)
            level_out(0, [acc_lo[:], acc_hi[:]], cnt0[:])

    nc.compile()
    return nc


def _host_fallback(scr_bi, fmap_b, h, k, off):
    """Feature at argmax of the soft mask; only used when a mask is empty."""
    V = scr_bi[off::k, :][:h].astype(np.float32) + scr_bi[off + 1 :: k, :][:h]
    sr4 = V[:, off::k][:, :h] + V[:, off + 1 :: k][:, :h]
    idx = int(np.argmax(np.float32(0.25) * sr4))
    y, x = divmod(idx, h)
    return fmap_b[:, y, x]


def _host_cnt(scribbles):
    """Exact per-level foreground-pixel counts, [B, I, 3].  The kernel's mask
    math is bit-identical, so any mismatch means the device returned garbage
    (wedged NeuronCore) and the run must be retried."""
    out = np.empty((scribbles.shape[0], scribbles.shape[1], 3), np.float32)
    for li, (h, k, off) in _LEVELS.items():
        V = (
            scribbles[:, :, off::k, :][:, :, :h].astype(np.float32)
            + scribbles[:, :, off + 1 :: k, :][:, :, :h]
        )
        S = V[:, :, :, off::k][:, :, :, :h] + V[:, :, :, off + 1 :: k][:, :, :, :h]
        out[:, :, li] = (S > np.float32(2.0)).sum(axis=(-1, -2))
    return out


def kernel(feat0, feat1, feat2, scribbles):
    import sys

    for p in ("/opt/trn_rl_repo", "/opt/pypackages"):
        if p not in sys.path:
            sys.path.append(p)
    from concourse.bass_utils import run_bass_kernel_spmd

    feat0 = np.asarray(feat0, dtype=np.float32)
    feat1 = np.asarray(feat1, dtype=np.float32)
    feat2 = np.asarray(feat2, dtype=np.float32)
    scribbles = np.asarray(scribbles, dtype=np.float32)

    nc = _build_nc()
    in_maps = [
        {
            "feat0": np.ascontiguousarray(feat0[b]),
            "feat1": np.ascontiguousarray(feat1[b]),
            "feat2": np.ascontiguousarray(feat2[b]),
            "scribbles": np.ascontiguousarray(scribbles[b]),
        }
        for b in range(_B)
    ]
    want_cnt = _host_cnt(scribbles)
    for attempt in range(3):
        res = run_bass_kernel_spmd(nc, in_maps, core_ids=list(range(_B)))
        raw = np.stack([res.results[b]["out"] for b in range(_B)])  # [B,I,3*257]
        raw = raw.reshape(_B, _I, 3, _C + 1)
        ssum = raw[..., :_C].astype(np.float32)  # [B, I, 3, C]
        cnt = raw[..., _C].astype(np.float32)  # [B, I, 3]
        if np.array_equal(cnt, want_cnt) and np.isfinite(ssum).all():
            break
        # device returned garbage (wedged core) -- rebuild and retry
        nc = _build_nc()

    mean = ssum / np.maximum(cnt, np.float32(1.0))[..., None]

    if (cnt == 0).any():  # never for non-degenerate inputs
        fm = [feat0, feat1, feat2]
        for b, i, li in zip(*np.nonzero(cnt == 0)):
            h, k, off = _LEVELS[li]
            mean[b, i, li] = _host_fallback(scribbles[b, i], fm[li][b], h, k, off)

    out = (mean[:, :, 0] + mean[:, :, 1] + mean[:, :, 2]) / np.float32(3.0)
    return out.astype(np.float32)
